# revision 29
# baseline (speedup 1.0000x reference)
"""Distributed Trainium2 (8 NeuronCores) causal self-attention block.

Reference computation:
    qkv = x @ w_attn + b_attn            # [B,S,3D]
    q,k,v = split heads                  # [B,H,S,HD]
    present = stack([k,v])               # [B,2,H,S,HD]
    w = softmax(mask(q k^T / sqrt(HD)))  # causal
    a = (w @ v) merged                   # [B,S,D]
    out = a @ w_proj + b_proj

B=4, S=2048, D=1024, H=16, HD=64 -> 8192 tokens, 64 (b,h) pairs.

Sharding: tensor-parallel over heads; core c owns heads {2c, 2c+1}.
  - x / weights are cast to bf16 on the host; all PE operands are bf16
    (pipelined LDWEIGHTS + FWL), PSUM accumulation stays fp32.
  - x is transposed on the host (input prep) so x^T tiles stream
    in with plain contiguous DMAs.
  - column-parallel QKV produces q^T,k^T (bf16, [cols, tokens]) and a
    transposed v; attention per (b, head) uses transposed scores
    st[key, q], est = exp(st/8) (no max-subtraction needed; masked
    lanes multiply to exactly 0 via bf16 causal masks), PV via
    ones-augmented v so the softmax denominator falls out of the same
    accumulation.
  - per-batch AllToAll re-shards a^T (bf16) from head-split to
    token-split; the projection computes this core's token rows
    against the full w_proj. Host assembles full outputs.
"""

import sys

sys.path.insert(0, "/opt/trn_rl_repo")

import numpy as np

import concourse.bass as bass
import concourse.mybir as mybir
import concourse.tile as tile
from concourse import bacc
from concourse.bass_utils import run_bass_kernel_spmd
from concourse.masks import make_identity

F32 = mybir.dt.float32
BF16 = mybir.dt.bfloat16
AF = mybir.ActivationFunctionType
ALU = mybir.AluOpType

B, S, D, H, HD = 4, 2048, 1024, 16, 64
NCORES = 8
TOK = B * S  # 8192
P = 128
QB = 512  # query block (free dim of st / pv matmuls)
NJ = S // QB  # 4 q-blocks per batch
NKT = S // P  # 16 key tiles per batch
TBLK = 512  # token block in QKV phase
NTB_B = S // TBLK  # 4 token blocks per batch


def _build():
    nc = bacc.Bacc(
        "TRN2", target_bir_lowering=False, debug=False, num_devices=NCORES
    )
    xt_d = nc.dram_tensor("xT", [D, TOK], BF16, kind="ExternalInput")
    wqkv_d = nc.dram_tensor("wqkv", [D, 384], BF16, kind="ExternalInput")
    bqkv_d = nc.dram_tensor("bqkv", [384, 1], F32, kind="ExternalInput")
    wp_d = nc.dram_tensor("w_proj", [D, D], BF16, kind="ExternalInput")
    bp_d = nc.dram_tensor("b_proj", [1, D], F32, kind="ExternalInput")
    out_d = nc.dram_tensor("out_part", [1024, D], F32, kind="ExternalOutput")
    pres_d = nc.dram_tensor(
        "present_part", [B, 2, 2, S, HD], BF16, kind="ExternalOutput"
    )

    rg = [list(range(NCORES))]

    with tile.TileContext(nc) as tc:
        # ---------------- constants ----------------
        const = tc.alloc_tile_pool(name="const", bufs=1)
        ident = const.tile([P, P], BF16, tag="ident", name="ident")
        make_identity(nc, ident[:])

        # 4 diagonal causal masks [key=128, q=512]: 1 where q-k-off>=0
        masks = []
        for dof in range(4):
            m = const.tile([P, QB], BF16, tag=f"mask{dof}", name=f"mask{dof}")
            nc.gpsimd.memset(m[:], 1.0)
            nc.gpsimd.affine_select(
                out=m[:],
                in_=m[:],
                pattern=[[1, QB]],
                compare_op=ALU.is_ge,
                fill=0.0,
                base=-(P * dof),
                channel_multiplier=-1,
            )
            masks.append(m)

        ones_col = const.tile([P, 1], BF16, tag="ones_col", name="ones_col")
        nc.gpsimd.memset(ones_col[:], 1.0)

        wq_sb = const.tile([P, 8 * 384], BF16, tag="wq_sb", name="wq_sb")
        for k in range(8):
            nc.sync.dma_start(
                wq_sb[:, 384 * k : 384 * (k + 1)],
                wqkv_d.ap()[P * k : P * (k + 1), :],
            )
        wp_sb = const.tile([P, 8 * 1024], BF16, tag="wp_sb", name="wp_sb")
        for k in range(8):
            nc.sync.dma_start(
                wp_sb[:, 1024 * k : 1024 * (k + 1)],
                wp_d.ap()[P * k : P * (k + 1), :],
            )
        bq_sb = const.tile([P, 3], F32, tag="bq_sb", name="bq_sb")
        for c in range(3):
            nc.sync.dma_start(
                bq_sb[:, c : c + 1], bqkv_d.ap()[P * c : P * (c + 1), :]
            )
        bp_row = const.tile([1, D], F32, tag="bp_row", name="bp_row")
        nc.sync.dma_start(bp_row[:], bp_d.ap()[:, :])
        bp_sb = const.tile([P, D], F32, tag="bp_sb", name="bp_sb")
        nc.gpsimd.partition_broadcast(bp_sb[:], bp_row[:])

        # PE warm-up primer: ~5us of dependency-free back-to-back matmuls
        # right at kernel start so the HAM clock-gate opens (1.2 -> 2.4 GHz)
        # before the real matmul stream begins.
        warm_sb = const.tile([P, P], BF16, tag="warm_sb", name="warm_sb")
        nc.gpsimd.memset(warm_sb[:], 0.0)

        # per-batch activation tiles, double-buffered across batches
        qkt_pool = tc.alloc_tile_pool(name="qkt_pool", bufs=2)
        qT, kT, vA, vB = {}, {}, {}, {}

        # ---------------- transient pools ----------------
        xt_pool = tc.alloc_tile_pool(name="xt_pool", bufs=10)
        vt_pool = tc.alloc_tile_pool(name="vt_pool", bufs=2)
        kn_pool = tc.alloc_tile_pool(name="kn_pool", bufs=3)
        est_pool = tc.alloc_tile_pool(name="est_pool", bufs=8)
        at_pool = tc.alloc_tile_pool(name="at_pool", bufs=3)
        den_pool = tc.alloc_tile_pool(name="den_pool", bufs=4)
        osb_pool = tc.alloc_tile_pool(name="osb_pool", bufs=2)
        plhs_pool = tc.alloc_tile_pool(name="plhs_pool", bufs=10)

        tp_psum = tc.alloc_tile_pool(name="tp_psum", bufs=2, space="PSUM")
        st_psum = tc.alloc_tile_pool(name="st_psum", bufs=2, space="PSUM")
        pv_psum = tc.alloc_tile_pool(name="pv_psum", bufs=2, space="PSUM")

        dram = tc.alloc_tile_pool(name="dram", bufs=1, space="DRAM")
        warm_ps = tp_psum.tile([P, P], F32, tag="tp", name="warm_ps")
        for _ in range(48):
            nc.tensor.matmul(warm_ps[:], warm_sb[:], warm_sb[:], start=True, stop=True)
        nc.vector.tensor_copy(warm_sb[:], warm_ps[:])
        warm_dram = dram.tile([P, P], BF16, tag="warm_dram", name="warm_dram")
        nc.sync.dma_start(warm_dram[:], warm_sb[:])
        a2a_in = [
            dram.tile([NCORES, P, 256], BF16, tag=f"a2ain{b}", name=f"a2ain{b}")
            for b in range(B)
        ]
        a2a_out = [
            dram.tile([NCORES, P, 256], BF16, tag=f"a2aout{b}", name=f"a2aout{b}")
            for b in range(B)
        ]

        for b in range(B):
            # ======== QKV phase for batch b ========
            qT[b] = qkt_pool.tile([P, S], BF16, tag="qT", name="qT")
            kT[b] = qkt_pool.tile([P, S], BF16, tag="kT", name="kT")
            # v natural, ones-augmented: 16 chunks of [128 keys, 65]
            vA[b] = qkt_pool.tile([P, 65 * NKT], BF16, tag="vA", name="vA")
            vB[b] = qkt_pool.tile([P, 65 * NKT], BF16, tag="vB", name="vB")
            xts_b = []
            for k in range(8):
                xtt = xt_pool.tile([P, S], BF16, tag="xt", name="xt")
                nc.sync.dma_start(
                    xtt[:], xt_d.ap()[P * k : P * (k + 1), b * S : (b + 1) * S]
                )
                xts_b.append(xtt)
            for tb in range(NTB_B):
                tok0 = b * S + tb * TBLK
                xts = [xtt[:, tb * TBLK : (tb + 1) * TBLK] for xtt in xts_b]
                for c in range(3):
                    ps = tp_psum.tile([P, TBLK], F32, tag="tp", name="qkvp")
                    for k in range(8):
                        nc.tensor.matmul(
                            ps[:],
                            wq_sb[:, 384 * k + P * c : 384 * k + P * (c + 1)],
                            xts[k],
                            start=(k == 0),
                            stop=(k == 7),
                        )
                    tsl = slice(tb * TBLK, (tb + 1) * TBLK)
                    if c == 0:
                        nc.vector.tensor_scalar_add(
                            qT[b][:, tsl], ps[:], bq_sb[:, 0:1]
                        )
                    elif c == 1:
                        nc.vector.tensor_scalar_add(
                            kT[b][:, tsl], ps[:], bq_sb[:, 1:2]
                        )
                        # present-k: transpose back to [tokens, hd]
                        for s in range(4):
                            ptk = tp_psum.tile([P, P], BF16, tag="tp", name="ptk")
                            nc.tensor.transpose(
                                ptk[:],
                                kT[b][:, tb * TBLK + P * s : tb * TBLK + P * (s + 1)],
                                ident[:],
                            )
                            kn = kn_pool.tile([P, P], BF16, tag="kn", name="kn")
                            nc.vector.tensor_copy(kn[:], ptk[:])
                            s0 = tb * TBLK + P * s
                            nc.gpsimd.dma_start(
                                pres_d.ap()[b, 0, 0, s0 : s0 + P, :], kn[:, 0:HD]
                            )
                            nc.gpsimd.dma_start(
                                pres_d.ap()[b, 0, 1, s0 : s0 + P, :], kn[:, HD:P]
                            )
                    else:
                        vt = vt_pool.tile([P, TBLK], BF16, tag="vt", name="vt")
                        nc.vector.tensor_scalar_add(vt[:], ps[:], bq_sb[:, 2:3])
                        for s in range(4):
                            ptv = tp_psum.tile([P, P], BF16, tag="tp", name="ptv")
                            nc.tensor.transpose(
                                ptv[:], vt[:, P * s : P * (s + 1)], ident[:]
                            )
                            t = tb * 4 + s  # key-chunk index within batch
                            nc.vector.tensor_copy(
                                vA[b][:, 65 * t : 65 * t + HD], ptv[:, 0:HD]
                            )
                            nc.vector.tensor_copy(
                                vB[b][:, 65 * t : 65 * t + HD], ptv[:, HD:P]
                            )
                            if b < 2:  # slots rotate with bufs=2
                                nc.vector.tensor_copy(
                                    vA[b][:, 65 * t + HD : 65 * t + HD + 1],
                                    ones_col[:],
                                )
                                nc.vector.tensor_copy(
                                    vB[b][:, 65 * t + HD : 65 * t + HD + 1],
                                    ones_col[:],
                                )
                            s0v = t * P
                            nc.gpsimd.dma_start(
                                pres_d.ap()[b, 1, 0, s0v : s0v + P, :],
                                vA[b][:, 65 * t : 65 * t + HD],
                            )
                            nc.gpsimd.dma_start(
                                pres_d.ap()[b, 1, 1, s0v : s0v + P, :],
                                vB[b][:, 65 * t : 65 * t + HD],
                            )

            # ======== attention phase for batch b ========
            for j in range(NJ):
                q0 = QB * j
                nkt = 4 * j + 4  # valid key tiles (causal)
                pvA = pv_psum.tile([65, QB], F32, tag="pv", name="pvA")
                pvB = pv_psum.tile([65, QB], F32, tag="pv", name="pvB")
                for i in range(nkt):
                    k0 = P * i
                    dof = i - 4 * j
                    # causal: q-columns below 128*dof are entirely invalid
                    f0 = P * dof if dof > 0 else 0
                    nq = QB - f0
                    stAB = st_psum.tile([P, 2 * QB], F32, tag="st", name="stAB")
                    nc.tensor.matmul(
                        stAB[:, f0:QB],
                        kT[b][0:HD, k0 : k0 + P],
                        qT[b][0:HD, q0 + f0 : q0 + QB],
                        start=True,
                        stop=True,
                        tile_position=(0, 0),
                    )
                    nc.tensor.matmul(
                        stAB[:, QB + f0 : 2 * QB],
                        kT[b][HD:P, k0 : k0 + P],
                        qT[b][HD:P, q0 + f0 : q0 + QB],
                        start=True,
                        stop=True,
                        tile_position=(HD, 0),
                    )
                    eAB = est_pool.tile([P, 2 * QB], BF16, tag="est", name="est")
                    if dof > 0:
                        nc.scalar.activation(
                            eAB[:, f0:QB], stAB[:, f0:QB], AF.Exp, scale=0.125
                        )
                        nc.scalar.activation(
                            eAB[:, QB + f0 : 2 * QB],
                            stAB[:, QB + f0 : 2 * QB],
                            AF.Exp,
                            scale=0.125,
                        )
                    else:
                        nc.scalar.activation(
                            eAB[:], stAB[:], AF.Exp, scale=0.125
                        )
                    if dof >= 0:  # diagonal-crossing tile
                        nc.vector.tensor_tensor(
                            eAB[:, f0:QB],
                            eAB[:, f0:QB],
                            masks[dof][:, f0:QB],
                            ALU.mult,
                        )
                        nc.vector.tensor_tensor(
                            eAB[:, QB + f0 : 2 * QB],
                            eAB[:, QB + f0 : 2 * QB],
                            masks[dof][:, f0:QB],
                            ALU.mult,
                        )
                    nc.tensor.matmul(
                        pvA[:, f0:QB],
                        vA[b][:, 65 * i : 65 * (i + 1)],
                        eAB[:, f0:QB],
                        start=(i == 0),
                        stop=(i == nkt - 1),
                    )
                    nc.tensor.matmul(
                        pvB[:, f0:QB],
                        vB[b][:, 65 * i : 65 * (i + 1)],
                        eAB[:, QB + f0 : 2 * QB],
                        start=(i == 0),
                        stop=(i == nkt - 1),
                    )
                aT = at_pool.tile([P, QB], BF16, tag="aT", name="aT")
                for pv, r0 in ((pvA, 0), (pvB, HD)):
                    dsb = den_pool.tile([1, QB], F32, tag="den", name="den")
                    nc.vector.tensor_copy(dsb[:], pv[HD : HD + 1, :])
                    rsb = den_pool.tile([1, QB], F32, tag="rden", name="rden")
                    nc.vector.reciprocal_approx_fast(out=rsb[:], in_=dsb[:])
                    rb = den_pool.tile([HD, QB], F32, tag="rdenb", name="rdenb")
                    nc.gpsimd.partition_broadcast(rb[:], rsb[:])
                    nc.vector.tensor_tensor(
                        aT[r0 : r0 + HD, :], pv[0:HD, :], rb[:], ALU.mult
                    )
                nc.sync.dma_start(a2a_in[b][2 * j, :, :], aT[:, 0:256])
                nc.sync.dma_start(a2a_in[b][2 * j + 1, :, :], aT[:, 256:512])

            # ======== resharding collective for batch b ========
            nc.gpsimd.collective_compute(
                "AllToAll",
                ALU.bypass,
                replica_groups=rg,
                ins=[a2a_in[b][:].opt()],
                outs=[a2a_out[b][:].opt()],
            )

            # ======== projection for batch b (256 tokens of this core) ====
            for m in range(2):
                lts = []
                for k in range(8):
                    lt = plhs_pool.tile([P, P], BF16, tag="plhs", name="plhs")
                    nc.sync.dma_start(
                        lt[:], a2a_out[b][k, :, P * m : P * (m + 1)]
                    )
                    lts.append(lt)
                for n in range(2):
                    ps = pv_psum.tile([P, 512], F32, tag="pv", name="pjp")
                    for k in range(8):
                        nc.tensor.matmul(
                            ps[:],
                            lts[k][:],
                            wp_sb[:, 1024 * k + 512 * n : 1024 * k + 512 * (n + 1)],
                            start=(k == 0),
                            stop=(k == 7),
                        )
                    osb = osb_pool.tile([P, 512], F32, tag="osb", name="osb")
                    nc.vector.tensor_tensor(
                        osb[:], ps[:], bp_sb[:, 512 * n : 512 * (n + 1)], ALU.add
                    )
                    nc.sync.dma_start(
                        out_d.ap()[
                            256 * b + P * m : 256 * b + P * (m + 1),
                            512 * n : 512 * (n + 1),
                        ],
                        osb[:],
                    )

        plhs_pool.release()
        osb_pool.release()
        den_pool.release()
        at_pool.release()
        est_pool.release()
        kn_pool.release()
        vt_pool.release()
        xt_pool.release()
        qkt_pool.release()
        dram.release()
        pv_psum.release()
        st_psum.release()
        tp_psum.release()
        const.release()

    nc.finalize()
    return nc


_CACHE = {}


def _get_nc():
    if "nc" not in _CACHE:
        _CACHE["nc"] = _build()
    return _CACHE["nc"]


def _in_maps(x, w_attn, b_attn, w_proj, b_proj):
    import ml_dtypes

    bf16 = ml_dtypes.bfloat16
    xT = np.ascontiguousarray(
        np.asarray(x, np.float32).reshape(TOK, D).T.astype(bf16)
    )
    w_attn = np.asarray(w_attn, np.float32)
    b_attn = np.asarray(b_attn, np.float32)
    w_proj = np.ascontiguousarray(np.asarray(w_proj, np.float32).astype(bf16))
    b_proj = np.ascontiguousarray(
        np.asarray(b_proj, np.float32).reshape(1, D)
    )
    maps = []
    for c in range(NCORES):
        cols = slice(P * c, P * (c + 1))
        wq = np.concatenate(
            [
                w_attn[:, 0:1024][:, cols],
                w_attn[:, 1024:2048][:, cols],
                w_attn[:, 2048:3072][:, cols],
            ],
            axis=1,
        ).astype(bf16)
        bq = np.concatenate(
            [b_attn[0:1024][cols], b_attn[1024:2048][cols], b_attn[2048:3072][cols]]
        ).reshape(384, 1)
        maps.append(
            dict(
                xT=xT,
                wqkv=np.ascontiguousarray(wq),
                bqkv=np.ascontiguousarray(bq),
                w_proj=w_proj,
                b_proj=b_proj,
            )
        )
    return maps


def _assemble(results):
    a = np.empty((TOK, D), np.float32)
    present = np.empty((B, 2, H, S, HD), np.float32)
    for c in range(NCORES):
        op = results[c]["out_part"]
        for b in range(B):
            a[S * b + 256 * c : S * b + 256 * (c + 1)] = op[256 * b : 256 * (b + 1)]
        present[:, :, 2 * c : 2 * c + 2] = np.asarray(
            results[c]["present_part"], np.float32
        )
    return a.reshape(B, S, D), present


def run(trace=False, **inputs):
    nc = _get_nc()
    res = run_bass_kernel_spmd(
        nc,
        _in_maps(**inputs),
        core_ids=list(range(NCORES)),
        trace=trace,
    )
    return _assemble(res.results), res


def kernel(**inputs):
    out, _ = run(trace=False, **inputs)
    return out


# revision 30
# speedup vs baseline: 1.0394x; 1.0394x over previous
"""Distributed Trainium2 (8 NeuronCores) causal self-attention block.

Reference computation:
    qkv = x @ w_attn + b_attn            # [B,S,3D]
    q,k,v = split heads                  # [B,H,S,HD]
    present = stack([k,v])               # [B,2,H,S,HD]
    w = softmax(mask(q k^T / sqrt(HD)))  # causal
    a = (w @ v) merged                   # [B,S,D]
    out = a @ w_proj + b_proj

B=4, S=2048, D=1024, H=16, HD=64 -> 8192 tokens, 64 (b,h) pairs.

Sharding: tensor-parallel over heads; core c owns heads {2c, 2c+1}.
  - x / weights are cast to bf16 on the host; all PE operands are bf16
    (pipelined LDWEIGHTS + FWL), PSUM accumulation stays fp32.
  - x^T tiles come from hardware DMA-transpose (2-byte path).
  - column-parallel QKV produces q^T,k^T (bf16, [cols, tokens]) and a
    transposed v; attention per (b, head) uses transposed scores
    st[key, q], est = exp(st/8) (no max-subtraction needed; masked
    lanes multiply to exactly 0 via bf16 causal masks), PV via
    ones-augmented v so the softmax denominator falls out of the same
    accumulation.
  - per-batch AllToAll re-shards a^T (bf16) from head-split to
    token-split; the projection computes this core's token rows
    against the full w_proj. Host assembles full outputs.
"""

import sys

sys.path.insert(0, "/opt/trn_rl_repo")

import numpy as np

import concourse.bass as bass
import concourse.mybir as mybir
import concourse.tile as tile
from concourse import bacc
from concourse.bass_utils import run_bass_kernel_spmd
from concourse.masks import make_identity

F32 = mybir.dt.float32
BF16 = mybir.dt.bfloat16
AF = mybir.ActivationFunctionType
ALU = mybir.AluOpType

B, S, D, H, HD = 4, 2048, 1024, 16, 64
NCORES = 8
TOK = B * S  # 8192
P = 128
QB = 512  # query block (free dim of st / pv matmuls)
NJ = S // QB  # 4 q-blocks per batch
NKT = S // P  # 16 key tiles per batch
TBLK = 512  # token block in QKV phase
NTB_B = S // TBLK  # 4 token blocks per batch


def _build():
    nc = bacc.Bacc(
        "TRN2", target_bir_lowering=False, debug=False, num_devices=NCORES
    )
    xt_d = nc.dram_tensor("xT", [D, TOK], BF16, kind="ExternalInput")
    wqkv_d = nc.dram_tensor("wqkv", [D, 384], BF16, kind="ExternalInput")
    bqkv_d = nc.dram_tensor("bqkv", [384, 1], F32, kind="ExternalInput")
    wp_d = nc.dram_tensor("w_proj", [D, D], BF16, kind="ExternalInput")
    bp_d = nc.dram_tensor("b_proj", [1, D], F32, kind="ExternalInput")
    out_d = nc.dram_tensor("out_part", [1024, D], F32, kind="ExternalOutput")
    pres_d = nc.dram_tensor(
        "present_part", [B, 2, 2, S, HD], BF16, kind="ExternalOutput"
    )

    rg = [list(range(NCORES))]

    with tile.TileContext(nc) as tc:
        # ---------------- constants ----------------
        const = tc.alloc_tile_pool(name="const", bufs=1)
        ident = const.tile([P, P], BF16, tag="ident", name="ident")
        make_identity(nc, ident[:])

        # 4 diagonal causal masks [key=128, q=512]: 1 where q-k-off>=0
        masks = []
        for dof in range(4):
            m = const.tile([P, QB], BF16, tag=f"mask{dof}", name=f"mask{dof}")
            nc.gpsimd.memset(m[:], 1.0)
            nc.gpsimd.affine_select(
                out=m[:],
                in_=m[:],
                pattern=[[1, QB]],
                compare_op=ALU.is_ge,
                fill=0.0,
                base=-(P * dof),
                channel_multiplier=-1,
            )
            masks.append(m)

        ones_col = const.tile([P, 1], BF16, tag="ones_col", name="ones_col")
        nc.gpsimd.memset(ones_col[:], 1.0)

        wq_sb = const.tile([P, 8 * 384], BF16, tag="wq_sb", name="wq_sb")
        for k in range(8):
            nc.sync.dma_start(
                wq_sb[:, 384 * k : 384 * (k + 1)],
                wqkv_d.ap()[P * k : P * (k + 1), :],
            )
        wp_sb = const.tile([P, 8 * 1024], BF16, tag="wp_sb", name="wp_sb")
        for k in range(8):
            nc.sync.dma_start(
                wp_sb[:, 1024 * k : 1024 * (k + 1)],
                wp_d.ap()[P * k : P * (k + 1), :],
            )
        bq_sb = const.tile([P, 3], F32, tag="bq_sb", name="bq_sb")
        for c in range(3):
            nc.sync.dma_start(
                bq_sb[:, c : c + 1], bqkv_d.ap()[P * c : P * (c + 1), :]
            )
        bp_row = const.tile([1, D], F32, tag="bp_row", name="bp_row")
        nc.sync.dma_start(bp_row[:], bp_d.ap()[:, :])
        bp_sb = const.tile([P, D], F32, tag="bp_sb", name="bp_sb")
        nc.gpsimd.partition_broadcast(bp_sb[:], bp_row[:])

        # PE warm-up primer: ~5us of dependency-free back-to-back matmuls
        # right at kernel start so the HAM clock-gate opens (1.2 -> 2.4 GHz)
        # before the real matmul stream begins.
        warm_sb = const.tile([P, P], BF16, tag="warm_sb", name="warm_sb")
        nc.gpsimd.memset(warm_sb[:], 0.0)

        # per-batch activation tiles, double-buffered across batches
        qkt_pool = tc.alloc_tile_pool(name="qkt_pool", bufs=2)
        qT, kT, vA, vB = {}, {}, {}, {}

        # ---------------- transient pools ----------------
        xt_pool = tc.alloc_tile_pool(name="xt_pool", bufs=12)
        vt_pool = tc.alloc_tile_pool(name="vt_pool", bufs=2)
        kn_pool = tc.alloc_tile_pool(name="kn_pool", bufs=3)
        est_pool = tc.alloc_tile_pool(name="est_pool", bufs=8)
        at_pool = tc.alloc_tile_pool(name="at_pool", bufs=3)
        den_pool = tc.alloc_tile_pool(name="den_pool", bufs=4)
        osb_pool = tc.alloc_tile_pool(name="osb_pool", bufs=2)
        plhs_pool = tc.alloc_tile_pool(name="plhs_pool", bufs=10)

        tp_psum = tc.alloc_tile_pool(name="tp_psum", bufs=2, space="PSUM")
        st_psum = tc.alloc_tile_pool(name="st_psum", bufs=2, space="PSUM")
        pv_psum = tc.alloc_tile_pool(name="pv_psum", bufs=2, space="PSUM")

        dram = tc.alloc_tile_pool(name="dram", bufs=1, space="DRAM")
        warm_ps = tp_psum.tile([P, P], F32, tag="tp", name="warm_ps")
        for _ in range(48):
            nc.tensor.matmul(warm_ps[:], warm_sb[:], warm_sb[:], start=True, stop=True)
        nc.vector.tensor_copy(warm_sb[:], warm_ps[:])
        warm_dram = dram.tile([P, P], BF16, tag="warm_dram", name="warm_dram")
        nc.sync.dma_start(warm_dram[:], warm_sb[:])
        a2a_in = [
            dram.tile([NCORES, P, 256], BF16, tag=f"a2ain{b}", name=f"a2ain{b}")
            for b in range(B)
        ]
        a2a_out = [
            dram.tile([NCORES, P, 256], BF16, tag=f"a2aout{b}", name=f"a2aout{b}")
            for b in range(B)
        ]

        for b in range(B):
            # ======== QKV phase for batch b ========
            qT[b] = qkt_pool.tile([P, S], BF16, tag="qT", name="qT")
            kT[b] = qkt_pool.tile([P, S], BF16, tag="kT", name="kT")
            # v natural, ones-augmented: 16 chunks of [128 keys, 65]
            vA[b] = qkt_pool.tile([P, 65 * NKT], BF16, tag="vA", name="vA")
            vB[b] = qkt_pool.tile([P, 65 * NKT], BF16, tag="vB", name="vB")
            for tb in range(NTB_B):
                tok0 = b * S + tb * TBLK
                xts = []
                for k in range(8):
                    xt = xt_pool.tile([P, TBLK], BF16, tag="xt", name="xt")
                    nc.sync.dma_start(
                        xt[:],
                        xt_d.ap()[P * k : P * (k + 1), tok0 : tok0 + TBLK],
                    )
                    xts.append(xt)
                for c in range(3):
                    ps = tp_psum.tile([P, TBLK], F32, tag="tp", name="qkvp")
                    for k in range(8):
                        nc.tensor.matmul(
                            ps[:],
                            wq_sb[:, 384 * k + P * c : 384 * k + P * (c + 1)],
                            xts[k][:],
                            start=(k == 0),
                            stop=(k == 7),
                        )
                    tsl = slice(tb * TBLK, (tb + 1) * TBLK)
                    if c == 0:
                        nc.vector.tensor_scalar_add(
                            qT[b][:, tsl], ps[:], bq_sb[:, 0:1]
                        )
                    elif c == 1:
                        nc.vector.tensor_scalar_add(
                            kT[b][:, tsl], ps[:], bq_sb[:, 1:2]
                        )
                        # present-k: transpose back to [tokens, hd]
                        for s in range(4):
                            ptk = tp_psum.tile([P, P], BF16, tag="tp", name="ptk")
                            nc.tensor.transpose(
                                ptk[:],
                                kT[b][:, tb * TBLK + P * s : tb * TBLK + P * (s + 1)],
                                ident[:],
                            )
                            kn = kn_pool.tile([P, P], BF16, tag="kn", name="kn")
                            nc.vector.tensor_copy(kn[:], ptk[:])
                            s0 = tb * TBLK + P * s
                            nc.gpsimd.dma_start(
                                pres_d.ap()[b, 0, 0, s0 : s0 + P, :], kn[:, 0:HD]
                            )
                            nc.gpsimd.dma_start(
                                pres_d.ap()[b, 0, 1, s0 : s0 + P, :], kn[:, HD:P]
                            )
                    else:
                        vt = vt_pool.tile([P, TBLK], BF16, tag="vt", name="vt")
                        nc.vector.tensor_scalar_add(vt[:], ps[:], bq_sb[:, 2:3])
                        for s in range(4):
                            ptv = tp_psum.tile([P, P], BF16, tag="tp", name="ptv")
                            nc.tensor.transpose(
                                ptv[:], vt[:, P * s : P * (s + 1)], ident[:]
                            )
                            t = tb * 4 + s  # key-chunk index within batch
                            nc.vector.tensor_copy(
                                vA[b][:, 65 * t : 65 * t + HD], ptv[:, 0:HD]
                            )
                            nc.vector.tensor_copy(
                                vB[b][:, 65 * t : 65 * t + HD], ptv[:, HD:P]
                            )
                            if b < 2:  # slots rotate with bufs=2
                                nc.vector.tensor_copy(
                                    vA[b][:, 65 * t + HD : 65 * t + HD + 1],
                                    ones_col[:],
                                )
                                nc.vector.tensor_copy(
                                    vB[b][:, 65 * t + HD : 65 * t + HD + 1],
                                    ones_col[:],
                                )
                            s0v = t * P
                            nc.gpsimd.dma_start(
                                pres_d.ap()[b, 1, 0, s0v : s0v + P, :],
                                vA[b][:, 65 * t : 65 * t + HD],
                            )
                            nc.gpsimd.dma_start(
                                pres_d.ap()[b, 1, 1, s0v : s0v + P, :],
                                vB[b][:, 65 * t : 65 * t + HD],
                            )

            # ======== attention phase for batch b ========
            for j in range(NJ):
                q0 = QB * j
                nkt = 4 * j + 4  # valid key tiles (causal)
                pvA = pv_psum.tile([65, QB], F32, tag="pv", name="pvA")
                pvB = pv_psum.tile([65, QB], F32, tag="pv", name="pvB")
                for i in range(nkt):
                    k0 = P * i
                    dof = i - 4 * j
                    # causal: q-columns below 128*dof are entirely invalid
                    f0 = P * dof if dof > 0 else 0
                    nq = QB - f0
                    stAB = st_psum.tile([P, 2 * QB], F32, tag="st", name="stAB")
                    nc.tensor.matmul(
                        stAB[:, f0:QB],
                        kT[b][0:HD, k0 : k0 + P],
                        qT[b][0:HD, q0 + f0 : q0 + QB],
                        start=True,
                        stop=True,
                        tile_position=(0, 0),
                    )
                    nc.tensor.matmul(
                        stAB[:, QB + f0 : 2 * QB],
                        kT[b][HD:P, k0 : k0 + P],
                        qT[b][HD:P, q0 + f0 : q0 + QB],
                        start=True,
                        stop=True,
                        tile_position=(HD, 0),
                    )
                    eAB = est_pool.tile([P, 2 * QB], BF16, tag="est", name="est")
                    if dof > 0:
                        nc.scalar.activation(
                            eAB[:, f0:QB], stAB[:, f0:QB], AF.Exp, scale=0.125
                        )
                        nc.scalar.activation(
                            eAB[:, QB + f0 : 2 * QB],
                            stAB[:, QB + f0 : 2 * QB],
                            AF.Exp,
                            scale=0.125,
                        )
                    else:
                        nc.scalar.activation(
                            eAB[:], stAB[:], AF.Exp, scale=0.125
                        )
                    if dof >= 0:  # diagonal-crossing tile
                        nc.vector.tensor_tensor(
                            eAB[:, f0:QB],
                            eAB[:, f0:QB],
                            masks[dof][:, f0:QB],
                            ALU.mult,
                        )
                        nc.vector.tensor_tensor(
                            eAB[:, QB + f0 : 2 * QB],
                            eAB[:, QB + f0 : 2 * QB],
                            masks[dof][:, f0:QB],
                            ALU.mult,
                        )
                    nc.tensor.matmul(
                        pvA[:, f0:QB],
                        vA[b][:, 65 * i : 65 * (i + 1)],
                        eAB[:, f0:QB],
                        start=(i == 0),
                        stop=(i == nkt - 1),
                    )
                    nc.tensor.matmul(
                        pvB[:, f0:QB],
                        vB[b][:, 65 * i : 65 * (i + 1)],
                        eAB[:, QB + f0 : 2 * QB],
                        start=(i == 0),
                        stop=(i == nkt - 1),
                    )
                aT = at_pool.tile([P, QB], BF16, tag="aT", name="aT")
                for pv, r0 in ((pvA, 0), (pvB, HD)):
                    dsb = den_pool.tile([1, QB], F32, tag="den", name="den")
                    nc.vector.tensor_copy(dsb[:], pv[HD : HD + 1, :])
                    rsb = den_pool.tile([1, QB], F32, tag="rden", name="rden")
                    nc.vector.reciprocal_approx_fast(out=rsb[:], in_=dsb[:])
                    rb = den_pool.tile([HD, QB], F32, tag="rdenb", name="rdenb")
                    nc.gpsimd.partition_broadcast(rb[:], rsb[:])
                    nc.vector.tensor_tensor(
                        aT[r0 : r0 + HD, :], pv[0:HD, :], rb[:], ALU.mult
                    )
                nc.sync.dma_start(a2a_in[b][2 * j, :, :], aT[:, 0:256])
                nc.sync.dma_start(a2a_in[b][2 * j + 1, :, :], aT[:, 256:512])

            # ======== resharding collective for batch b ========
            nc.gpsimd.collective_compute(
                "AllToAll",
                ALU.bypass,
                replica_groups=rg,
                ins=[a2a_in[b][:].opt()],
                outs=[a2a_out[b][:].opt()],
            )

            # ======== projection for batch b (256 tokens of this core) ====
            for m in range(2):
                lts = []
                for k in range(8):
                    lt = plhs_pool.tile([P, P], BF16, tag="plhs", name="plhs")
                    nc.sync.dma_start(
                        lt[:], a2a_out[b][k, :, P * m : P * (m + 1)]
                    )
                    lts.append(lt)
                for n in range(2):
                    ps = pv_psum.tile([P, 512], F32, tag="pv", name="pjp")
                    for k in range(8):
                        nc.tensor.matmul(
                            ps[:],
                            lts[k][:],
                            wp_sb[:, 1024 * k + 512 * n : 1024 * k + 512 * (n + 1)],
                            start=(k == 0),
                            stop=(k == 7),
                        )
                    osb = osb_pool.tile([P, 512], F32, tag="osb", name="osb")
                    nc.vector.tensor_tensor(
                        osb[:], ps[:], bp_sb[:, 512 * n : 512 * (n + 1)], ALU.add
                    )
                    nc.sync.dma_start(
                        out_d.ap()[
                            256 * b + P * m : 256 * b + P * (m + 1),
                            512 * n : 512 * (n + 1),
                        ],
                        osb[:],
                    )

        plhs_pool.release()
        osb_pool.release()
        den_pool.release()
        at_pool.release()
        est_pool.release()
        kn_pool.release()
        vt_pool.release()
        xt_pool.release()
        qkt_pool.release()
        dram.release()
        pv_psum.release()
        st_psum.release()
        tp_psum.release()
        const.release()

    nc.finalize()
    return nc


_CACHE = {}


def _get_nc():
    if "nc" not in _CACHE:
        _CACHE["nc"] = _build()
    return _CACHE["nc"]


def _in_maps(x, w_attn, b_attn, w_proj, b_proj):
    import ml_dtypes

    bf16 = ml_dtypes.bfloat16
    xT = np.ascontiguousarray(
        np.asarray(x, np.float32).reshape(TOK, D).T.astype(bf16)
    )
    w_attn = np.asarray(w_attn, np.float32)
    b_attn = np.asarray(b_attn, np.float32)
    w_proj = np.ascontiguousarray(np.asarray(w_proj, np.float32).astype(bf16))
    b_proj = np.ascontiguousarray(
        np.asarray(b_proj, np.float32).reshape(1, D)
    )
    maps = []
    for c in range(NCORES):
        cols = slice(P * c, P * (c + 1))
        wq = np.concatenate(
            [
                w_attn[:, 0:1024][:, cols],
                w_attn[:, 1024:2048][:, cols],
                w_attn[:, 2048:3072][:, cols],
            ],
            axis=1,
        ).astype(bf16)
        bq = np.concatenate(
            [b_attn[0:1024][cols], b_attn[1024:2048][cols], b_attn[2048:3072][cols]]
        ).reshape(384, 1)
        maps.append(
            dict(
                xT=xT,
                wqkv=np.ascontiguousarray(wq),
                bqkv=np.ascontiguousarray(bq),
                w_proj=w_proj,
                b_proj=b_proj,
            )
        )
    return maps


def _assemble(results):
    a = np.empty((TOK, D), np.float32)
    present = np.empty((B, 2, H, S, HD), np.float32)
    for c in range(NCORES):
        op = results[c]["out_part"]
        for b in range(B):
            a[S * b + 256 * c : S * b + 256 * (c + 1)] = op[256 * b : 256 * (b + 1)]
        present[:, :, 2 * c : 2 * c + 2] = np.asarray(
            results[c]["present_part"], np.float32
        )
    return a.reshape(B, S, D), present


def run(trace=False, **inputs):
    nc = _get_nc()
    res = run_bass_kernel_spmd(
        nc,
        _in_maps(**inputs),
        core_ids=list(range(NCORES)),
        trace=trace,
    )
    return _assemble(res.results), res


def kernel(**inputs):
    out, _ = run(trace=False, **inputs)
    return out


# revision 33
# speedup vs baseline: 1.0887x; 1.0474x over previous
"""Distributed Trainium2 (8 NeuronCores) causal self-attention block.

Reference computation:
    qkv = x @ w_attn + b_attn            # [B,S,3D]
    q,k,v = split heads                  # [B,H,S,HD]
    present = stack([k,v])               # [B,2,H,S,HD]
    w = softmax(mask(q k^T / sqrt(HD)))  # causal
    a = (w @ v) merged                   # [B,S,D]
    out = a @ w_proj + b_proj

B=4, S=2048, D=1024, H=16, HD=64 -> 8192 tokens, 64 (b,h) pairs.

Sharding: tensor-parallel over heads; core c owns heads {2c, 2c+1}.
  - x / weights are cast to bf16 on the host; all PE operands are bf16
    (pipelined LDWEIGHTS + FWL), PSUM accumulation stays fp32.
  - x is transposed on the host (input prep) so x^T tiles stream
    in with plain contiguous DMAs.
  - column-parallel QKV produces q^T,k^T (bf16, [cols, tokens]) and a
    transposed v; attention per (b, head) uses transposed scores
    st[key, q], est = exp(st/8) (no max-subtraction needed; masked
    lanes multiply to exactly 0 via bf16 causal masks), PV via
    ones-augmented v so the softmax denominator falls out of the same
    accumulation.
  - per-batch AllToAll re-shards a^T (bf16) from head-split to
    token-split; the projection computes this core's token rows
    against the full w_proj. Host assembles full outputs.
"""

import sys

sys.path.insert(0, "/opt/trn_rl_repo")

import numpy as np

import concourse.bass as bass
import concourse.mybir as mybir
import concourse.tile as tile
from concourse import bacc
from concourse.bass_utils import run_bass_kernel_spmd
from concourse.masks import make_identity

F32 = mybir.dt.float32
BF16 = mybir.dt.bfloat16
AF = mybir.ActivationFunctionType
ALU = mybir.AluOpType

B, S, D, H, HD = 4, 2048, 1024, 16, 64
NCORES = 8
TOK = B * S  # 8192
P = 128
QB = 512  # query block (free dim of st / pv matmuls)
NJ = S // QB  # 4 q-blocks per batch
NKT = S // P  # 16 key tiles per batch
TBLK = 512  # token block in QKV phase
NTB_B = S // TBLK  # 4 token blocks per batch


def _build():
    nc = bacc.Bacc(
        "TRN2", target_bir_lowering=False, debug=False, num_devices=NCORES
    )
    xt_d = nc.dram_tensor("xT", [D, TOK], BF16, kind="ExternalInput")
    wqkv_d = nc.dram_tensor("wqkv", [D, 384], BF16, kind="ExternalInput")
    bqkv_d = nc.dram_tensor("bqkv", [384, 1], F32, kind="ExternalInput")
    wp_d = nc.dram_tensor("w_proj", [D, D], BF16, kind="ExternalInput")
    bp_d = nc.dram_tensor("b_proj", [1, D], F32, kind="ExternalInput")
    out_d = nc.dram_tensor("out_part", [1024, D], F32, kind="ExternalOutput")
    pres_d = nc.dram_tensor(
        "present_part", [B, 2, 2, S, HD], BF16, kind="ExternalOutput"
    )

    rg = [list(range(NCORES))]

    with tile.TileContext(nc) as tc:
        # ---------------- constants ----------------
        const = tc.alloc_tile_pool(name="const", bufs=1)
        ident = const.tile([P, P], BF16, tag="ident", name="ident")
        make_identity(nc, ident[:])

        # 4 diagonal causal masks [key=128, q=512]: 1 where q-k-off>=0
        masks = []
        for dof in range(4):
            m = const.tile([P, QB], BF16, tag=f"mask{dof}", name=f"mask{dof}")
            nc.gpsimd.memset(m[:], 1.0)
            nc.gpsimd.affine_select(
                out=m[:],
                in_=m[:],
                pattern=[[1, QB]],
                compare_op=ALU.is_ge,
                fill=0.0,
                base=-(P * dof),
                channel_multiplier=-1,
            )
            masks.append(m)

        ones_col = const.tile([P, 1], BF16, tag="ones_col", name="ones_col")
        nc.gpsimd.memset(ones_col[:], 1.0)

        wq_sb = const.tile([P, 8 * 384], BF16, tag="wq_sb", name="wq_sb")
        for k in range(8):
            nc.sync.dma_start(
                wq_sb[:, 384 * k : 384 * (k + 1)],
                wqkv_d.ap()[P * k : P * (k + 1), :],
            )
        wp_sb = const.tile([P, 8 * 1024], BF16, tag="wp_sb", name="wp_sb")
        for k in range(8):
            nc.sync.dma_start(
                wp_sb[:, 1024 * k : 1024 * (k + 1)],
                wp_d.ap()[P * k : P * (k + 1), :],
            )
        bq_sb = const.tile([P, 3], F32, tag="bq_sb", name="bq_sb")
        for c in range(3):
            nc.sync.dma_start(
                bq_sb[:, c : c + 1], bqkv_d.ap()[P * c : P * (c + 1), :]
            )
        bp_row = const.tile([1, D], F32, tag="bp_row", name="bp_row")
        nc.sync.dma_start(bp_row[:], bp_d.ap()[:, :])
        bp_sb = const.tile([P, D], F32, tag="bp_sb", name="bp_sb")
        nc.gpsimd.partition_broadcast(bp_sb[:], bp_row[:])

        # PE warm-up primer: ~5us of dependency-free back-to-back matmuls
        # right at kernel start so the HAM clock-gate opens (1.2 -> 2.4 GHz)
        # before the real matmul stream begins.
        warm_sb = const.tile([P, P], BF16, tag="warm_sb", name="warm_sb")
        nc.gpsimd.memset(warm_sb[:], 0.0)

        # per-batch activation tiles, double-buffered across batches
        qkt_pool = tc.alloc_tile_pool(name="qkt_pool", bufs=3)
        qT, kT, vA, vB = {}, {}, {}, {}

        # ---------------- transient pools ----------------
        xt_pool = tc.alloc_tile_pool(name="xt_pool", bufs=12)
        vt_pool = tc.alloc_tile_pool(name="vt_pool", bufs=2)
        kn_pool = tc.alloc_tile_pool(name="kn_pool", bufs=3)
        est_pool = tc.alloc_tile_pool(name="est_pool", bufs=10)
        at_pool = tc.alloc_tile_pool(name="at_pool", bufs=3)
        den_pool = tc.alloc_tile_pool(name="den_pool", bufs=4)
        osb_pool = tc.alloc_tile_pool(name="osb_pool", bufs=2)
        plhs_pool = tc.alloc_tile_pool(name="plhs_pool", bufs=10)

        tp_psum = tc.alloc_tile_pool(name="tp_psum", bufs=2, space="PSUM")
        st_psum = tc.alloc_tile_pool(name="st_psum", bufs=2, space="PSUM")
        pv_psum = tc.alloc_tile_pool(name="pv_psum", bufs=2, space="PSUM")

        dram = tc.alloc_tile_pool(name="dram", bufs=1, space="DRAM")
        warm_ps = tp_psum.tile([P, P], F32, tag="tp", name="warm_ps")
        for _ in range(48):
            nc.tensor.matmul(warm_ps[:], warm_sb[:], warm_sb[:], start=True, stop=True)
        nc.vector.tensor_copy(warm_sb[:], warm_ps[:])
        warm_dram = dram.tile([P, P], BF16, tag="warm_dram", name="warm_dram")
        nc.sync.dma_start(warm_dram[:], warm_sb[:])
        a2a_in = [
            dram.tile([NCORES, P, 256], BF16, tag=f"a2ain{b}", name=f"a2ain{b}")
            for b in range(B)
        ]
        a2a_out = [
            dram.tile([NCORES, P, 256], BF16, tag=f"a2aout{b}", name=f"a2aout{b}")
            for b in range(B)
        ]

        for b in range(B):
            # ======== QKV phase for batch b ========
            qT[b] = qkt_pool.tile([P, S], BF16, tag="qT", name="qT")
            kT[b] = qkt_pool.tile([P, S], BF16, tag="kT", name="kT")
            # v natural, ones-augmented: 16 chunks of [128 keys, 65]
            vA[b] = qkt_pool.tile([P, 65 * NKT], BF16, tag="vA", name="vA")
            vB[b] = qkt_pool.tile([P, 65 * NKT], BF16, tag="vB", name="vB")
            for tb in range(NTB_B):
                tok0 = b * S + tb * TBLK
                xts = []
                for k in range(8):
                    xt = xt_pool.tile([P, TBLK], BF16, tag="xt", name="xt")
                    nc.sync.dma_start(
                        xt[:],
                        xt_d.ap()[P * k : P * (k + 1), tok0 : tok0 + TBLK],
                    )
                    xts.append(xt)
                for c in range(3):
                    ps = tp_psum.tile([P, TBLK], F32, tag="tp", name="qkvp")
                    for k in range(8):
                        nc.tensor.matmul(
                            ps[:],
                            wq_sb[:, 384 * k + P * c : 384 * k + P * (c + 1)],
                            xts[k][:],
                            start=(k == 0),
                            stop=(k == 7),
                        )
                    tsl = slice(tb * TBLK, (tb + 1) * TBLK)
                    if c == 0:
                        nc.vector.tensor_scalar_add(
                            qT[b][:, tsl], ps[:], bq_sb[:, 0:1]
                        )
                    elif c == 1:
                        nc.vector.tensor_scalar_add(
                            kT[b][:, tsl], ps[:], bq_sb[:, 1:2]
                        )
                        # present-k: transpose back to [tokens, hd]
                        for s in range(4):
                            ptk = tp_psum.tile([P, P], BF16, tag="tp", name="ptk")
                            nc.tensor.transpose(
                                ptk[:],
                                kT[b][:, tb * TBLK + P * s : tb * TBLK + P * (s + 1)],
                                ident[:],
                            )
                            kn = kn_pool.tile([P, P], BF16, tag="kn", name="kn")
                            nc.vector.tensor_copy(kn[:], ptk[:])
                            s0 = tb * TBLK + P * s
                            nc.gpsimd.dma_start(
                                pres_d.ap()[b, 0, 0, s0 : s0 + P, :], kn[:, 0:HD]
                            )
                            nc.gpsimd.dma_start(
                                pres_d.ap()[b, 0, 1, s0 : s0 + P, :], kn[:, HD:P]
                            )
                    else:
                        vt = vt_pool.tile([P, TBLK], BF16, tag="vt", name="vt")
                        nc.vector.tensor_scalar_add(vt[:], ps[:], bq_sb[:, 2:3])
                        for s in range(4):
                            ptv = tp_psum.tile([P, P], BF16, tag="tp", name="ptv")
                            nc.tensor.transpose(
                                ptv[:], vt[:, P * s : P * (s + 1)], ident[:]
                            )
                            t = tb * 4 + s  # key-chunk index within batch
                            nc.vector.tensor_copy(
                                vA[b][:, 65 * t : 65 * t + HD], ptv[:, 0:HD]
                            )
                            nc.vector.tensor_copy(
                                vB[b][:, 65 * t : 65 * t + HD], ptv[:, HD:P]
                            )
                            if b < 3:  # slots rotate with qkt_pool bufs
                                nc.vector.tensor_copy(
                                    vA[b][:, 65 * t + HD : 65 * t + HD + 1],
                                    ones_col[:],
                                )
                                nc.vector.tensor_copy(
                                    vB[b][:, 65 * t + HD : 65 * t + HD + 1],
                                    ones_col[:],
                                )
                            s0v = t * P
                            nc.gpsimd.dma_start(
                                pres_d.ap()[b, 1, 0, s0v : s0v + P, :],
                                vA[b][:, 65 * t : 65 * t + HD],
                            )
                            nc.gpsimd.dma_start(
                                pres_d.ap()[b, 1, 1, s0v : s0v + P, :],
                                vB[b][:, 65 * t : 65 * t + HD],
                            )

            # ======== attention phase for batch b ========
            for j in range(NJ):
                q0 = QB * j
                nkt = 4 * j + 4  # valid key tiles (causal)
                pvA = pv_psum.tile([65, QB], F32, tag="pv", name="pvA")
                pvB = pv_psum.tile([65, QB], F32, tag="pv", name="pvB")
                for i in range(nkt):
                    k0 = P * i
                    dof = i - 4 * j
                    # causal: q-columns below 128*dof are entirely invalid
                    f0 = P * dof if dof > 0 else 0
                    nq = QB - f0
                    stAB = st_psum.tile([P, 2 * QB], F32, tag="st", name="stAB")
                    nc.tensor.matmul(
                        stAB[:, f0:QB],
                        kT[b][0:HD, k0 : k0 + P],
                        qT[b][0:HD, q0 + f0 : q0 + QB],
                        start=True,
                        stop=True,
                        tile_position=(0, 0),
                    )
                    nc.tensor.matmul(
                        stAB[:, QB + f0 : 2 * QB],
                        kT[b][HD:P, k0 : k0 + P],
                        qT[b][HD:P, q0 + f0 : q0 + QB],
                        start=True,
                        stop=True,
                        tile_position=(HD, 0),
                    )
                    eAB = est_pool.tile([P, 2 * QB], BF16, tag="est", name="est")
                    if dof > 0:
                        nc.scalar.activation(
                            eAB[:, f0:QB], stAB[:, f0:QB], AF.Exp, scale=0.125
                        )
                        nc.scalar.activation(
                            eAB[:, QB + f0 : 2 * QB],
                            stAB[:, QB + f0 : 2 * QB],
                            AF.Exp,
                            scale=0.125,
                        )
                    else:
                        nc.scalar.activation(
                            eAB[:], stAB[:], AF.Exp, scale=0.125
                        )
                    if dof >= 0:  # diagonal-crossing tile
                        nc.vector.tensor_tensor(
                            eAB[:, f0:QB],
                            eAB[:, f0:QB],
                            masks[dof][:, f0:QB],
                            ALU.mult,
                        )
                        nc.vector.tensor_tensor(
                            eAB[:, QB + f0 : 2 * QB],
                            eAB[:, QB + f0 : 2 * QB],
                            masks[dof][:, f0:QB],
                            ALU.mult,
                        )
                    nc.tensor.matmul(
                        pvA[:, f0:QB],
                        vA[b][:, 65 * i : 65 * (i + 1)],
                        eAB[:, f0:QB],
                        start=(i == 0),
                        stop=(i == nkt - 1),
                    )
                    nc.tensor.matmul(
                        pvB[:, f0:QB],
                        vB[b][:, 65 * i : 65 * (i + 1)],
                        eAB[:, QB + f0 : 2 * QB],
                        start=(i == 0),
                        stop=(i == nkt - 1),
                    )
                aT = at_pool.tile([P, QB], BF16, tag="aT", name="aT")
                for pv, r0 in ((pvA, 0), (pvB, HD)):
                    dsb = den_pool.tile([1, QB], F32, tag="den", name="den")
                    nc.vector.tensor_copy(dsb[:], pv[HD : HD + 1, :])
                    rsb = den_pool.tile([1, QB], F32, tag="rden", name="rden")
                    nc.vector.reciprocal_approx_fast(out=rsb[:], in_=dsb[:])
                    rb = den_pool.tile([HD, QB], F32, tag="rdenb", name="rdenb")
                    nc.gpsimd.partition_broadcast(rb[:], rsb[:])
                    nc.vector.tensor_tensor(
                        aT[r0 : r0 + HD, :], pv[0:HD, :], rb[:], ALU.mult
                    )
                nc.sync.dma_start(a2a_in[b][2 * j, :, :], aT[:, 0:256])
                nc.sync.dma_start(a2a_in[b][2 * j + 1, :, :], aT[:, 256:512])

            # ======== resharding collective for batch b ========
            nc.gpsimd.collective_compute(
                "AllToAll",
                ALU.bypass,
                replica_groups=rg,
                ins=[a2a_in[b][:].opt()],
                outs=[a2a_out[b][:].opt()],
            )

            # ======== projection for batch b (256 tokens of this core) ====
            for m in range(2):
                lts = []
                for k in range(8):
                    lt = plhs_pool.tile([P, P], BF16, tag="plhs", name="plhs")
                    nc.sync.dma_start(
                        lt[:], a2a_out[b][k, :, P * m : P * (m + 1)]
                    )
                    lts.append(lt)
                for n in range(2):
                    ps = pv_psum.tile([P, 512], F32, tag="pv", name="pjp")
                    for k in range(8):
                        nc.tensor.matmul(
                            ps[:],
                            lts[k][:],
                            wp_sb[:, 1024 * k + 512 * n : 1024 * k + 512 * (n + 1)],
                            start=(k == 0),
                            stop=(k == 7),
                        )
                    osb = osb_pool.tile([P, 512], F32, tag="osb", name="osb")
                    nc.vector.tensor_tensor(
                        osb[:], ps[:], bp_sb[:, 512 * n : 512 * (n + 1)], ALU.add
                    )
                    nc.sync.dma_start(
                        out_d.ap()[
                            256 * b + P * m : 256 * b + P * (m + 1),
                            512 * n : 512 * (n + 1),
                        ],
                        osb[:],
                    )

        plhs_pool.release()
        osb_pool.release()
        den_pool.release()
        at_pool.release()
        est_pool.release()
        kn_pool.release()
        vt_pool.release()
        xt_pool.release()
        qkt_pool.release()
        dram.release()
        pv_psum.release()
        st_psum.release()
        tp_psum.release()
        const.release()

    nc.finalize()
    return nc


_CACHE = {}


def _get_nc():
    if "nc" not in _CACHE:
        _CACHE["nc"] = _build()
    return _CACHE["nc"]


def _in_maps(x, w_attn, b_attn, w_proj, b_proj):
    import ml_dtypes

    bf16 = ml_dtypes.bfloat16
    xT = np.ascontiguousarray(
        np.asarray(x, np.float32).reshape(TOK, D).T.astype(bf16)
    )
    w_attn = np.asarray(w_attn, np.float32)
    b_attn = np.asarray(b_attn, np.float32)
    w_proj = np.ascontiguousarray(np.asarray(w_proj, np.float32).astype(bf16))
    b_proj = np.ascontiguousarray(
        np.asarray(b_proj, np.float32).reshape(1, D)
    )
    maps = []
    for c in range(NCORES):
        cols = slice(P * c, P * (c + 1))
        wq = np.concatenate(
            [
                w_attn[:, 0:1024][:, cols],
                w_attn[:, 1024:2048][:, cols],
                w_attn[:, 2048:3072][:, cols],
            ],
            axis=1,
        ).astype(bf16)
        bq = np.concatenate(
            [b_attn[0:1024][cols], b_attn[1024:2048][cols], b_attn[2048:3072][cols]]
        ).reshape(384, 1)
        maps.append(
            dict(
                xT=xT,
                wqkv=np.ascontiguousarray(wq),
                bqkv=np.ascontiguousarray(bq),
                w_proj=w_proj,
                b_proj=b_proj,
            )
        )
    return maps


def _assemble(results):
    a = np.empty((TOK, D), np.float32)
    present = np.empty((B, 2, H, S, HD), np.float32)
    for c in range(NCORES):
        op = results[c]["out_part"]
        for b in range(B):
            a[S * b + 256 * c : S * b + 256 * (c + 1)] = op[256 * b : 256 * (b + 1)]
        present[:, :, 2 * c : 2 * c + 2] = np.asarray(
            results[c]["present_part"], np.float32
        )
    return a.reshape(B, S, D), present


def run(trace=False, **inputs):
    nc = _get_nc()
    res = run_bass_kernel_spmd(
        nc,
        _in_maps(**inputs),
        core_ids=list(range(NCORES)),
        trace=trace,
    )
    return _assemble(res.results), res


def kernel(**inputs):
    out, _ = run(trace=False, **inputs)
    return out
